# revision 8
# baseline (speedup 1.0000x reference)
# Trainium2 Bass kernel for DnCNN+S4D (nn_DnCNN_S4_74182675137230).
#
# Data parallel over batch B=64 across 8 NeuronCores (BL=8 per core).
# The S4D FFT long-conv is computed exactly via a chunked state-space scan
# (chunk C=128, stride-2 carry).  Per channel h with per-chunk notation
# u_e[m] = chunk 2m, u_o[m] = chunk 2m+1 (m = 0..7):
#
#   sle[m] = V @ u_e[m]                       (A-phase matmul)
#   pi[m]  = ZV @ u_e[m] + V @ u_o[m]         (A-phase matmul, ZV = diag(w^C)V)
#   c[0]   = pi[0];  c[m] = z2 (.) c[m-1] + pi[m]   (DVE scan, z2 = w^2C)
#   q[0]   = sle[0]; q[m] = sle[m] + z1 (.) c[m-1]  (DVE cmul,  z1 = w^C)
#   y      = T @ u + M @ qc                   (B-phase matmul)
#     where qc = [q0, c0, q1, c1, ..., c6, q7] so chunk k's cross term is
#     M @ qc[k-1], one 120-column matmul per channel.
# then gelu, channel-mix Wout (+bout +residual).  All matmuls fp16 with
# fp32 PSUM accumulation; T/V/ZV/M are weight-only host preprocessing.
# Training-mode BN statistics are AllReduced across the 8 cores (with a
# warmup collective at kernel start); the i2-fold and channel broadcast of
# the BN scale/bias use small fp32 matmuls instead of round-trip DMAs.
# Conv1 uses a host-built im2col (6-row stationary); conv9/16/17 use
# block-diagonal tap stationaries with parity-edge corrections.
# Orientation transposes are xbar DMAs split per batch so they pipeline
# with the producing/consuming compute.
#
# Layouts (i2 = chunk parity, ip = chunk pair, l = (2*ip+i2)*128 + c):
#   h-orient: [(i2,h)=128 part, (b=8, ip=8, c=128) free]   (convs, Wout, BN)
#   c-orient: [c=128 part, (b, ip, (i2,h)=128) free]       (per-h S4 matmuls)
#   state:    [(ri,n)=128 part, ...] re in partitions 0-63, im in 64-127

import numpy as np

import concourse.bass as bass
import concourse.bacc as bacc
import concourse.tile as tile
from concourse import mybir
from concourse.bass_utils import run_bass_kernel_spmd

F32 = mybir.dt.float32
F16 = mybir.dt.float16
AF = mybir.ActivationFunctionType
OP = mybir.AluOpType

NCORES = 8
B, H, N, L, NB = 64, 64, 64, 2048, 13
BL = B // NCORES          # 8 local batches
C = 128                   # chunk length
NCH = L // C              # 16 chunks
IP = NCH // 2             # 8 chunk pairs
KAP = 256.0               # state scaling to keep fp16 range
EPS = 1e-5
H2 = 2 * H                # 128 = (i2, h) partition extent
N2 = 2 * N                # 128 = (re/im, n) state extent
NSRC = 2 * IP - 1         # 15 qc source slots
HQ = 16                   # h per weight-stream quarter


# ---------------------------------------------------------------------------
# Host-side weight preprocessing (numpy) -> fp16 device matrices
# ---------------------------------------------------------------------------

def _host_prep(inputs):
    out = {}
    log_dt = np.asarray(inputs['s4_log_dt'], np.float64)
    logA_re = np.asarray(inputs['s4_logA_re'], np.float64)
    A_im = np.asarray(inputs['s4_A_im'], np.float64)
    C_re = np.asarray(inputs['s4_C_re'], np.float64)
    C_im = np.asarray(inputs['s4_C_im'], np.float64)
    D = np.asarray(inputs['s4_D'], np.float64)
    Wout = np.asarray(inputs['s4_Wout'], np.float64)
    bout = np.asarray(inputs['s4_bout'], np.float64)

    dt = np.exp(log_dt)[:, :, None]
    A = -np.exp(logA_re) + 1j * A_im
    dtA = dt * A
    w = np.exp(dtA)                                            # (NB,H,N)
    Ct = (C_re + 1j * C_im) * (np.exp(dtA) - 1.0) / A

    cc = np.arange(C)
    P = w[..., None] ** np.arange(2 * C + 1)                   # (NB,H,N,2C+1)
    K = 2.0 * np.real(np.einsum('jhn,jhne->jhe', Ct, P[..., :C]))
    K[:, :, 0] += D                                            # D*u folded

    # T lhsT [c', (h, c)] with T[c,c'] = K[c-c']
    dmat = cc[None, :] - cc[:, None]                           # (c',c)
    Tl = np.where((dmat >= 0)[None, None],
                  np.take_along_axis(np.broadcast_to(K[:, :, None, :],
                                                     (NB, H, C, C)),
                                     np.clip(dmat, 0, C - 1)[None, None],
                                     axis=3), 0.0)             # (NB,H,c',c)
    out['tmat'] = np.ascontiguousarray(
        Tl.transpose(2, 0, 1, 3).reshape(C, NB, H * C), np.float16)

    # V lhsT [c', (h, 2n)]: V[(ri,n),c'] = [Re;Im](Ct w^(C-1-c'))/KAP
    VC = Ct[..., None] * P[..., (C - 1) - cc]                  # (NB,H,N,c')
    Vl = np.concatenate([VC.real, VC.imag], axis=2) / KAP      # (NB,H,2N,c')
    out['vmat'] = np.ascontiguousarray(
        Vl.transpose(3, 0, 1, 2).reshape(C, NB, H * N2), np.float16)

    # ZV lhsT: same with an extra w^C factor (state pushed one more chunk)
    zC = w ** C
    VCz = zC[..., None] * VC
    ZVl = np.concatenate([VCz.real, VCz.imag], axis=2) / KAP
    out['zvmat'] = np.ascontiguousarray(
        ZVl.transpose(3, 0, 1, 2).reshape(C, NB, H * N2), np.float16)

    # M lhsT [(ri,n), (h, c)]: y_cross[c] = 2*KAP*[Re|-Im](w^{c+1}) . s
    MC = P[..., cc + 1]                                        # (NB,H,N,c)
    Ml = np.concatenate([2 * KAP * MC.real, -2 * KAP * MC.imag], axis=2)
    out['mmat'] = np.ascontiguousarray(
        Ml.transpose(0, 2, 1, 3).reshape(NB, N2, H * C).transpose(1, 0, 2),
        np.float16)

    # complex multipliers, BL-broadcast: tiles [(ri,n), NB, H, BL]
    def cmul_tiles(z):
        zr = np.broadcast_to(z.real.transpose(0, 2, 1)[:, :, :, None],
                             (NB, N, H, BL))
        zi = np.broadcast_to(z.imag.transpose(0, 2, 1)[:, :, :, None],
                             (NB, N, H, BL))
        r = np.concatenate([zr, zr], 1).transpose(1, 0, 2, 3)
        i = np.concatenate([zi, -zi], 1).transpose(1, 0, 2, 3)
        return (np.ascontiguousarray(r, np.float16),
                np.ascontiguousarray(i, np.float16))
    out['z2r'], out['z2i'] = cmul_tiles(w ** (2 * C))
    out['z1r'], out['z1i'] = cmul_tiles(zC)

    # Wout block-diag over i2: lhsT[(i2,h),(i2',o)] = Wout[o,h] d_{i2,i2'}
    wblk = np.zeros((NB, H2, H2))
    WT = Wout.transpose(0, 2, 1)
    wblk[:, :H, :H] = WT; wblk[:, H:, H:] = WT
    out['wblk'] = np.ascontiguousarray(wblk.transpose(1, 0, 2), np.float16)
    out['bout2'] = np.ascontiguousarray(
        np.concatenate([bout, bout], 1).T, np.float32)         # [128, NB]

    # conv stationaries (c9/c16/c17): per tap k block-diag over i2, plus
    # i2-crossing edge stationaries.
    def conv_stat(Wc):                                         # (O, Hin, 3)
        O = Wc.shape[0]; Hin = Wc.shape[1]
        res = {}
        for k in range(3):
            Wk = np.zeros((2 * Hin, 2 * O))
            Wk[:Hin, :O] = Wc[:, :, k].T; Wk[Hin:, O:] = Wc[:, :, k].T
            res[f'k{k}'] = Wk
        E0a = np.zeros((2 * Hin, 2 * O)); E0a[Hin:, :O] = Wc[:, :, 0].T
        E0b = np.zeros((2 * Hin, 2 * O)); E0b[:Hin, O:] = Wc[:, :, 0].T
        E2a = np.zeros((2 * Hin, 2 * O)); E2a[Hin:, :O] = Wc[:, :, 2].T
        E2b = np.zeros((2 * Hin, 2 * O)); E2b[:Hin, O:] = Wc[:, :, 2].T
        res.update(e0a=E0a, e0b=E0b, e2a=E2a, e2b=E2b)
        return {k_: v.astype(np.float16) for k_, v in res.items()}
    convall = np.zeros((128, 21, 128), np.float16)
    for ni, key in enumerate(('conv9_w', 'conv16_w', 'conv17_w')):
        cs = conv_stat(np.asarray(inputs[key], np.float64))
        for si, sfx in enumerate(('k0', 'k1', 'k2', 'e0a', 'e0b', 'e2a', 'e2b')):
            arr = cs[sfx]
            convall[:arr.shape[0], ni * 7 + si, :arr.shape[1]] = arr
    out['convall'] = np.ascontiguousarray(convall.reshape(128, 21 * 128))

    # conv1 im2col stationary [ (i2r, tap)=6, (i2, o)=128 ]
    W1 = np.asarray(inputs['conv1_w'], np.float64)             # (H, 1, 3)
    c1s = np.zeros((6, H2))
    for i2r in range(2):
        for tap in range(3):
            c1s[i2r * 3 + tap, i2r * H:(i2r + 1) * H] = W1[:, 0, tap]
    out['c1s'] = np.ascontiguousarray(c1s, np.float16)

    bn = np.zeros((H, 6), np.float32)
    for k, nm in enumerate(('bn1', 'bn9', 'bn16')):
        bn[:, 2 * k] = np.asarray(inputs[nm + '_g'], np.float32)
        bn[:, 2 * k + 1] = np.asarray(inputs[nm + '_b'], np.float32)
    out['bnp'] = bn

    # BN stat fold [128 -> 64] (mean of 16 core*half chunks) and channel
    # broadcast [64 -> 128] fp32 matrices
    ff = np.zeros((128, H), np.float32)
    ff[np.arange(128), np.arange(128) % H] = 1.0 / (2 * NCORES)
    out['ffold'] = ff
    f2 = np.zeros((H, 128), np.float32)
    f2[np.arange(128) % H, np.arange(128)] = 1.0
    out['f2'] = f2
    return out


def _host_prep_x(x_shard):
    # im2col for conv1: xim[(i2r,tap), b, ip, c] = x[b, (2ip+i2r)*128+c+tap-1]
    xs = np.asarray(x_shard, np.float32).reshape(BL, L)
    xp = np.zeros((BL, L + 2), np.float32)
    xp[:, 1:L + 1] = xs
    xim = np.zeros((6, BL, IP, C), np.float32)
    for i2r in range(2):
        for tap in range(3):
            sh = xp[:, tap:tap + L].reshape(BL, IP, 2, C)
            xim[i2r * 3 + tap] = sh[:, :, i2r, :]
    return np.ascontiguousarray(xim, np.float16)


# ---------------------------------------------------------------------------
# Device kernel
# ---------------------------------------------------------------------------

_CACHE = {}

CONV_SFX = ('k0', 'k1', 'k2', 'e0a', 'e0b', 'e2a', 'e2b')


def _bcast(zslice, n, at=1):
    """Insert a stride-0 axis of extent n into an AP after the partition."""
    ap = list(zslice.ap)
    ap2 = ap[:at] + [[0, n]] + ap[at:]
    return bass.AP(tensor=zslice.tensor, offset=zslice.offset, ap=ap2)


def _build():
    if 'nc' in _CACHE:
        return _CACHE['nc']
    import contextlib
    nc = bacc.Bacc("TRN2", target_bir_lowering=False, debug=False,
                   num_devices=NCORES)

    dram = {}
    def din(name, shape, dtype=F16):
        dram[name] = nc.dram_tensor(name, shape, dtype,
                                    kind="ExternalInput").ap()

    din('xim', [6, BL, IP, C])
    din('tmat', [C, NB, H * C]); din('vmat', [C, NB, H * N2])
    din('zvmat', [C, NB, H * N2]); din('mmat', [N2, NB, H * C])
    din('z2r', [N2, NB, H, BL]); din('z2i', [N2, NB, H, BL])
    din('z1r', [N2, NB, H, BL]); din('z1i', [N2, NB, H, BL])
    din('wblk', [H2, NB, H2]); din('bout2', [H2, NB], F32)
    din('convall', [128, 21 * 128])
    din('c1s', [6, H2])
    din('bnp', [H, 6], F32)
    din('ffold', [128, H], F32); din('f2', [H, 128], F32)
    out_d = nc.dram_tensor('out', [2, BL, IP, C], F32,
                           kind="ExternalOutput").ap()

    with tile.TileContext(nc) as tc:
        ctx = contextlib.ExitStack()
        P_stat = ctx.enter_context(tc.tile_pool(name="stat", bufs=1))
        P_act = ctx.enter_context(tc.tile_pool(name="act", bufs=1))
        P_wa = ctx.enter_context(tc.tile_pool(name="wa", bufs=3))
        P_wb = ctx.enter_context(tc.tile_pool(name="wb", bufs=3))
        P_z = ctx.enter_context(tc.tile_pool(name="wz", bufs=2))
        P_tmp = ctx.enter_context(tc.tile_pool(name="tmp", bufs=2))
        P_ps_a = ctx.enter_context(tc.tile_pool(name="psa", bufs=2,
                                                space="PSUM"))
        P_ps_b = ctx.enter_context(tc.tile_pool(name="psb", bufs=3,
                                                space="PSUM"))
        P_ps_w = ctx.enter_context(tc.tile_pool(name="psw", bufs=2,
                                                space="PSUM"))
        P_dram = ctx.enter_context(tc.tile_pool(name="cdram", bufs=1,
                                                space="DRAM"))

        # ---- warmup collective (pays one-time CC setup off-path) ----
        wu_in = P_dram.tile([128, 2], F32, tag="wuin")
        wu_out = P_dram.tile([128, 2], F32, tag="wuout", addr_space="Shared")
        wu_s = P_stat.tile([128, 2], F32, tag="wus")
        nc.vector.memset(wu_s, 0.0)
        nc.sync.dma_start(out=wu_in, in_=wu_s)
        nc.gpsimd.collective_compute("AllReduce", OP.add, ins=[wu_in[:]],
                                     outs=[wu_out[:]],
                                     replica_groups=[list(range(NCORES))])

        # ---- static tiles ----
        wblk_s = P_stat.tile([H2, NB, H2], F16)
        nc.sync.dma_start(out=wblk_s, in_=dram['wblk'])
        bout_s = P_stat.tile([H2, NB], F32)
        nc.sync.dma_start(out=bout_s, in_=dram['bout2'])
        bnp_s = P_stat.tile([H, 6], F32)
        nc.sync.dma_start(out=bnp_s, in_=dram['bnp'])
        xim_s = P_stat.tile([6, BL, IP, C], F16)
        nc.sync.dma_start(out=xim_s, in_=dram['xim'])
        c1s_s = P_stat.tile([6, H2], F16)
        nc.sync.dma_start(out=c1s_s, in_=dram['c1s'])
        ffold_s = P_stat.tile([128, H], F32)
        nc.sync.dma_start(out=ffold_s, in_=dram['ffold'])
        f2_s = P_stat.tile([H, 128], F32)
        nc.sync.dma_start(out=f2_s, in_=dram['f2'])
        convall_s = P_stat.tile([128, 21, 128], F16)
        nc.sync.dma_start(out=convall_s, in_=dram['convall'].rearrange(
            "p (k c) -> p k c", k=21))
        convw = {}
        for ni, nm in enumerate(('c9', 'c16', 'c17')):
            ncol = 2 if nm == 'c17' else H2
            convw[nm] = {sfx: convall_s[0:H2, ni * 7 + si, 0:ncol]
                         for si, sfx in enumerate(CONV_SFX)}

        # ---- activations ----
        u16c = P_act.tile([C, BL, IP, H2], F16, tag="u16c")
        g16c = P_act.tile([C, BL, IP, H2], F16, tag="g16c")
        gT = P_act.tile([H2, BL, IP, C], F16, tag="gT")
        uT = P_act.tile([H2, BL, IP, C], F16, tag="uT")
        uTn = P_act.tile([H2, BL, IP, C], F16, tag="uTn")
        sle_t = P_act.tile([N2, IP, H, BL], F16, tag="sle")
        pi_t = P_act.tile([N2, IP, H, BL], F16, tag="pi")
        qc_t = P_act.tile([N2, NSRC, H, BL], F16, tag="qc")
        gb128 = P_stat.tile([H2, 2], F32, tag="gb128")
        stats = P_stat.tile([H2, 16, 6], F32, tag="stats")
        bnscr = P_stat.tile([H2, 24], F32, tag="bnscr")

        u5 = u16c.rearrange("c b i (x h) -> c b i x h", x=2)
        qcp = qc_t[:, 1:NSRC].rearrange("n (m t) h b -> n m t h b", t=2)

        # ------------------------------------------------------------------
        def s4_block(j, u_h, u_next, last):
            z2r_s = P_z.tile([N2, H, BL], F16, tag="z2r")
            nc.sync.dma_start(out=z2r_s, in_=dram['z2r'][:, j])
            z2i_s = P_z.tile([N2, H, BL], F16, tag="z2i")
            nc.sync.dma_start(out=z2i_s, in_=dram['z2i'][:, j])
            z1r_s = P_z.tile([N2, H, BL], F16, tag="z1r")
            nc.sync.dma_start(out=z1r_s, in_=dram['z1r'][:, j])
            z1i_s = P_z.tile([N2, H, BL], F16, tag="z1i")
            nc.sync.dma_start(out=z1i_s, in_=dram['z1i'][:, j])

            # ---- A phase: sle + pi (V, ZV), 4h psum groups ----
            def a_phase(half):
                for q in (2 * half, 2 * half + 1):
                    hq = slice(HQ * q, HQ * (q + 1))
                    vm = P_wa.tile([C, HQ, N2], F16, tag="vm")
                    nc.sync.dma_start(
                        out=vm, in_=dram['vmat'][:, j].rearrange(
                            "p (h n) -> p h n", h=H)[:, hq, :])
                    zvm = P_wa.tile([C, HQ, N2], F16, tag="zvm")
                    nc.sync.dma_start(
                        out=zvm, in_=dram['zvmat'][:, j].rearrange(
                            "p (h n) -> p h n", h=H)[:, hq, :])
                    for g4 in range(HQ // 4):
                        acc = P_ps_a.tile([N2, 2, IP, 4, BL], F32,
                                          tag="apsum")
                        for hh in range(4):
                            hl = 4 * g4 + hh
                            h = HQ * q + hl
                            mv = u5[:, :, :, :, h].rearrange(
                                "c b i x -> c x i b")
                            nc.tensor.matmul(acc[:, :, :, hh, :],
                                             vm[:, hl, :], mv,
                                             start=True, stop=False)
                            mve = u5[:, :, :, 0, h].rearrange(
                                "c b i -> c i b")
                            nc.tensor.matmul(acc[:, 1, :, hh, :],
                                             zvm[:, hl, :], mve,
                                             start=False, stop=True)
                        hs4 = slice(HQ * q + 4 * g4, HQ * q + 4 * g4 + 4)
                        nc.scalar.activation(out=sle_t[:, :, hs4, :],
                                             in_=acc[:, 0], func=AF.Copy)
                        nc.vector.tensor_copy(out=pi_t[:, :, hs4, :],
                                              in_=acc[:, 1])

            # ---- DVE: carry scan + q construction per h-half ----
            def scan_phase(half):
                hs = slice(32 * half, 32 * (half + 1))
                nc.vector.tensor_copy(out=qc_t[:, 1, hs, :],
                                      in_=pi_t[:, 0, hs, :])
                nc.vector.tensor_copy(out=qc_t[:, 0, hs, :],
                                      in_=sle_t[:, 0, hs, :])
                z2r_h = z2r_s[:, hs, :]; z2i_h = z2i_s[:, hs, :]
                for m in range(1, IP - 1):
                    src = qc_t[:, 2 * m - 1, hs, :]
                    tsw = P_tmp.tile([N2, 32, BL], F16, tag="tsw")
                    nc.vector.tensor_tensor(out=tsw[0:N], in0=src[N:],
                                            in1=z2i_h[N:], op=OP.mult)
                    nc.vector.tensor_tensor(out=tsw[N:], in0=src[0:N],
                                            in1=z2i_h[0:N], op=OP.mult)
                    tzr = P_tmp.tile([N2, 32, BL], F16, tag="tzr")
                    nc.vector.tensor_tensor(out=tzr, in0=src, in1=z2r_h,
                                            op=OP.mult)
                    nc.vector.tensor_tensor(out=tzr, in0=tzr, in1=tsw,
                                            op=OP.add)
                    nc.vector.tensor_tensor(out=qc_t[:, 2 * m + 1, hs, :],
                                            in0=tzr, in1=pi_t[:, m, hs, :],
                                            op=OP.add)
                # q[m] = sle[m] + z1 (.) c[m-1], m = 1..7 in one sweep
                csrc = qcp[:, :, 0, hs, :]                     # c[0..6]
                qdst = qcp[:, :, 1, hs, :]                     # q[1..7]
                z1r_b = _bcast(z1r_s[:, hs, :], IP - 1)
                z1i_lo = _bcast(z1i_s[0:N, hs, :], IP - 1)
                z1i_hi = _bcast(z1i_s[N:, hs, :], IP - 1)
                tswq = P_tmp.tile([N2, IP - 1, 32, BL], F16, tag="tswq",
                                  bufs=1)
                nc.vector.tensor_tensor(out=tswq[0:N], in0=csrc[N:],
                                        in1=z1i_hi, op=OP.mult)
                nc.vector.tensor_tensor(out=tswq[N:], in0=csrc[0:N],
                                        in1=z1i_lo, op=OP.mult)
                tzrq = P_tmp.tile([N2, IP - 1, 32, BL], F16, tag="tzrq",
                                  bufs=1)
                nc.vector.tensor_tensor(out=tzrq, in0=csrc, in1=z1r_b,
                                        op=OP.mult)
                nc.vector.tensor_tensor(out=tzrq, in0=tzrq, in1=tswq,
                                        op=OP.add)
                nc.vector.tensor_tensor(out=qdst, in0=tzrq,
                                        in1=sle_t[:, 1:IP, hs, :], op=OP.add)

            # ---- B phase: y = T@u + M@qc, gelu evict ----
            def b_phase(half):
                for q in (2 * half, 2 * half + 1):
                    hq = slice(HQ * q, HQ * (q + 1))
                    tm = P_wb.tile([C, HQ, C], F16, tag="tm")
                    nc.sync.dma_start(
                        out=tm, in_=dram['tmat'][:, j].rearrange(
                            "p (h c) -> p h c", h=H)[:, hq, :])
                    mm = P_wb.tile([N2, HQ, C], F16, tag="mm")
                    nc.sync.dma_start(
                        out=mm, in_=dram['mmat'][:, j].rearrange(
                            "p (h c) -> p h c", h=H)[:, hq, :])
                    for g4 in range(HQ // 4):
                        acc = P_ps_b.tile([C, BL, IP, 2, 4], F32,
                                          tag="ypsum")
                        accf = acc.rearrange("c b i x h -> c b (i x) h")
                        for hh in range(4):
                            hl = 4 * g4 + hh
                            h = HQ * q + hl
                            nc.tensor.matmul(acc[:, :, :, :, hh],
                                             tm[:, hl, :],
                                             u5[:, :, :, :, h],
                                             start=True, stop=False)
                            qcm = qc_t[:, :, h, :].rearrange("n s b -> n b s")
                            nc.tensor.matmul(accf[:, :, 1:NCH, hh],
                                             mm[:, hl, :], qcm,
                                             start=False, stop=True)
                        sl = slice(HQ * q + 4 * g4, HQ * q + 4 * g4 + 4)
                        dst = g16c.rearrange(
                            "c b i (x h) -> c (b i) x h", x=2)[:, :, :, sl]
                        nc.scalar.activation(
                            out=dst,
                            in_=acc.rearrange("c b i x h -> c (b i) x h"),
                            func=AF.Gelu)

            a_phase(0)
            scan_phase(0)
            a_phase(1)
            scan_phase(1)
            b_phase(0)
            b_phase(1)

            # ---- per-batch: gT transpose, Wout + bout + residual, u16c ----
            gT_f = gT.rearrange("p b i c -> p (b i c)")
            uh_f = u_h.rearrange("p b i c -> p (b i c)")
            un_f = u_next.rearrange("p b i c -> p (b i c)")
            for b in range(BL):
                nc.sync.dma_start_transpose(
                    gT[:, b], g16c[:, b].rearrange("c i p -> c (i p)"))
                for t in range(2):
                    sl = slice(1024 * b + 512 * t, 1024 * b + 512 * (t + 1))
                    acc = P_ps_w.tile([H2, 512], F32, tag="wpsum")
                    nc.tensor.matmul(acc, wblk_s[:, j, :], gT_f[:, sl],
                                     start=True, stop=True)
                    nc.vector.scalar_tensor_tensor(
                        out=un_f[:, sl], in0=acc, scalar=bout_s[:, j:j + 1],
                        in1=uh_f[:, sl], op0=OP.add, op1=OP.add)
                if not last:
                    nc.sync.dma_start_transpose(
                        u16c[:, b], u_next[:, b].rearrange("p i c -> p (i c)"))

        # ------------------------------------------------------------------
        def conv_layer(src, w, big_out):
            """3-tap conv from h-orient src into gT raw + stats (big_out)
            or the final output DMA (conv17)."""
            for b in range(BL):
                for iph in range(2):
                    ip0 = 4 * iph
                    ips = slice(ip0, ip0 + 4)
                    acc = P_ps_b.tile([H2, 4, C], F32, tag="ypsum")
                    a = acc if big_out else acc[0:2]
                    nc.tensor.matmul(a, w['k1'], src[:, b, ips, :],
                                     start=True, stop=False)
                    nc.tensor.matmul(a[:, :, 1:C], w['k0'],
                                     src[:, b, ips, 0:C - 1],
                                     start=False, stop=False)
                    nc.tensor.matmul(a[:, :, 0:C - 1], w['k2'],
                                     src[:, b, ips, 1:C],
                                     start=False, stop=False)
                    if ip0 == 0:
                        nc.tensor.matmul(a[:, 1:4, 0:1], w['e0a'],
                                         src[:, b, 0:3, C - 1:C],
                                         start=False, stop=False)
                    else:
                        nc.tensor.matmul(a[:, 0:4, 0:1], w['e0a'],
                                         src[:, b, ip0 - 1:ip0 + 3, C - 1:C],
                                         start=False, stop=False)
                    nc.tensor.matmul(a[:, 0:4, 0:1], w['e0b'],
                                     src[:, b, ips, C - 1:C],
                                     start=False, stop=False)
                    nc.tensor.matmul(a[:, 0:4, C - 1:C], w['e2a'],
                                     src[:, b, ips, 0:1],
                                     start=False, stop=False)
                    if ip0 == 0:
                        nc.tensor.matmul(a[:, 0:4, C - 1:C], w['e2b'],
                                         src[:, b, 1:5, 0:1],
                                         start=False, stop=True)
                    else:
                        nc.tensor.matmul(a[:, 0:3, C - 1:C], w['e2b'],
                                         src[:, b, ip0 + 1:ip0 + 4, 0:1],
                                         start=False, stop=True)
                    if big_out:
                        nc.vector.bn_stats(out=stats[:, 2 * b + iph, :],
                                           in_=acc.rearrange(
                                               "p a c -> p (a c)"))
                        nc.scalar.activation(out=gT[:, b, ips, :], in_=acc,
                                             func=AF.Copy)
                    else:
                        ev = P_tmp.tile([2, 4, C], F32, tag="finev")
                        nc.scalar.activation(out=ev, in_=acc[0:2],
                                             func=AF.Copy)
                        nc.sync.dma_start(out=out_d[:, b, ips, :], in_=ev)

        def conv1_layer():
            for b in range(BL):
                for iph in range(2):
                    ips = slice(4 * iph, 4 * iph + 4)
                    acc = P_ps_b.tile([H2, 4, C], F32, tag="ypsum")
                    nc.tensor.matmul(acc, c1s_s, xim_s[:, b, ips, :],
                                     start=True, stop=True)
                    nc.vector.bn_stats(out=stats[:, 2 * b + iph, :],
                                       in_=acc.rearrange("p a c -> p (a c)"))
                    nc.scalar.activation(out=gT[:, b, ips, :], in_=acc,
                                         func=AF.Copy)

        def bn_finalize(k):
            """stats --bn_aggr/AllReduce/fold--> gb128 [128,2]=(g', b')."""
            mv = bnscr[:, 0:2]; pay = bnscr[:, 2:4]; red = bnscr[:, 4:6]
            nc.vector.bn_aggr(out=mv, in_=stats)
            nc.vector.tensor_tensor(out=pay[:, 1:2], in0=mv[:, 0:1],
                                    in1=mv[:, 0:1], op=OP.mult)
            nc.vector.tensor_tensor(out=pay[:, 1:2], in0=pay[:, 1:2],
                                    in1=mv[:, 1:2], op=OP.add)
            nc.vector.tensor_copy(out=pay[:, 0:1], in_=mv[:, 0:1])
            cin = P_dram.tile([H2, 2], F32, tag=f"bnin{k}")
            cout = P_dram.tile([H2, 2], F32, tag=f"bnout{k}",
                               addr_space="Shared")
            nc.sync.dma_start(out=cin, in_=pay)
            nc.gpsimd.collective_compute("AllReduce", OP.add, ins=[cin[:]],
                                         outs=[cout[:]],
                                         replica_groups=[list(range(NCORES))])
            nc.sync.dma_start(out=red, in_=cout)
            # fold i2-halves: gm [64, 2] = ffold.T @ red  (fp32 matmul)
            gacc = P_ps_w.tile([H, 2], F32, tag="wpsum")
            nc.tensor.matmul(gacc, ffold_s, red, start=True, stop=True)
            gm = bnscr[0:H, 6:8]
            nc.vector.tensor_copy(out=gm, in_=gacc)
            var = bnscr[0:H, 8:9]; eps_t = bnscr[0:H, 9:10]
            std = bnscr[0:H, 10:11]; rs = bnscr[0:H, 11:12]
            gp = bnscr[0:H, 12:14]
            nc.vector.tensor_tensor(out=var, in0=gm[:, 0:1], in1=gm[:, 0:1],
                                    op=OP.mult)
            nc.vector.tensor_tensor(out=var, in0=gm[:, 1:2], in1=var,
                                    op=OP.subtract)
            nc.vector.memset(eps_t, EPS)
            nc.scalar.activation(out=std, in_=var, func=AF.Sqrt, bias=eps_t)
            nc.vector.reciprocal(out=rs, in_=std)
            nc.vector.tensor_tensor(out=gp[:, 0:1],
                                    in0=bnp_s[:, 2 * k:2 * k + 1],
                                    in1=rs, op=OP.mult)
            nc.vector.tensor_tensor(out=gp[:, 1:2], in0=gp[:, 0:1],
                                    in1=gm[:, 0:1], op=OP.mult)
            nc.vector.tensor_tensor(out=gp[:, 1:2],
                                    in0=bnp_s[:, 2 * k + 1:2 * k + 2],
                                    in1=gp[:, 1:2], op=OP.subtract)
            # broadcast 64 -> 128 partitions: gb128 = f2.T @ gp
            bacc2 = P_ps_w.tile([H2, 2], F32, tag="wpsum")
            nc.tensor.matmul(bacc2, f2_s, gp, start=True, stop=True)
            nc.scalar.activation(out=gb128, in_=bacc2, func=AF.Copy)

        def bn_apply(dst):
            """dst = relu(g'*gT + b'), split scalar/vector, then u16c."""
            dst_f = dst.rearrange("p b i c -> p (b i c)")
            gT_ff = gT.rearrange("p b i c -> p (b i c)")
            for quad in range(4):
                sl = slice(2048 * quad, 2048 * (quad + 1))
                if quad % 2 == 0:
                    nc.scalar.activation(out=dst_f[:, sl], in_=gT_ff[:, sl],
                                         func=AF.Relu, bias=gb128[:, 1:2],
                                         scale=gb128[:, 0:1])
                else:
                    nc.vector.tensor_scalar(dst_f[:, sl], gT_ff[:, sl],
                                            gb128[:, 0:1], gb128[:, 1:2],
                                            OP.mult, OP.add)
                    nc.vector.tensor_scalar_max(dst_f[:, sl], dst_f[:, sl],
                                                0.0)
                for b in (2 * quad, 2 * quad + 1):
                    nc.sync.dma_start_transpose(
                        u16c[:, b], dst[:, b].rearrange("p i c -> p (i c)"))

        # ---------------- network ----------------
        conv1_layer()
        bn_finalize(0)
        bn_apply(uT)
        cur, nxt = uT, uTn
        for j in range(7):
            s4_block(j, cur, nxt, j == 6)
            cur, nxt = nxt, cur
        conv_layer(cur, convw['c9'], True)
        bn_finalize(1)
        bn_apply(cur)
        for j in range(7, 13):
            s4_block(j, cur, nxt, j == 12)
            cur, nxt = nxt, cur
        conv_layer(cur, convw['c16'], True)
        bn_finalize(2)
        bn_apply(cur)
        conv_layer(cur, convw['c17'], False)
        ctx.close()

    nc.compile()
    _CACHE['nc'] = nc
    return nc


# ---------------------------------------------------------------------------
# Entry point
# ---------------------------------------------------------------------------

def kernel(**inputs):
    nc = _build()
    prep = _host_prep(inputs)
    x = np.asarray(inputs['x'], np.float32)
    in_maps = []
    for c in range(NCORES):
        m = dict(prep)
        m['xim'] = _host_prep_x(x[c * BL:(c + 1) * BL])
        in_maps.append(m)
    res = run_bass_kernel_spmd(nc, in_maps, core_ids=list(range(NCORES)))
    outs = []
    for c in range(NCORES):
        o = res.results[c]['out']              # [2, BL, IP, C]
        outs.append(o.transpose(1, 2, 0, 3).reshape(BL, 1, L))
    return np.ascontiguousarray(np.concatenate(outs, 0), np.float32)


# revision 9
# speedup vs baseline: 1.0372x; 1.0372x over previous
# Trainium2 Bass kernel for DnCNN+S4D (nn_DnCNN_S4_74182675137230).
#
# Data parallel over batch B=64 across 8 NeuronCores (BL=8 per core).
# The S4D FFT long-conv is computed exactly via a chunked state-space scan
# (chunk C=128, stride-2 carry).  Per channel h with per-chunk notation
# u_e[m] = chunk 2m, u_o[m] = chunk 2m+1 (m = 0..7):
#
#   sle[m] = V @ u_e[m]                       (A-phase matmul)
#   pi[m]  = ZV @ u_e[m] + V @ u_o[m]         (A-phase matmul, ZV = diag(w^C)V)
#   c[0]   = pi[0];  c[m] = z2 (.) c[m-1] + pi[m]   (DVE scan, z2 = w^2C)
#   q[0]   = sle[0]; q[m] = sle[m] + z1 (.) c[m-1]  (DVE cmul,  z1 = w^C)
#   y      = T @ u + M @ qc                   (B-phase matmul)
#     where qc = [q0, c0, q1, c1, ..., c6, q7] so chunk k's cross term is
#     M @ qc[k-1], one 120-column matmul per channel.
# then gelu, channel-mix Wout (+bout +residual).  All matmuls fp16 with
# fp32 PSUM accumulation; T/V/ZV/M are weight-only host preprocessing.
# Training-mode BN statistics are AllReduced across the 8 cores (with a
# warmup collective at kernel start); the i2-fold and channel broadcast of
# the BN scale/bias use small fp32 matmuls instead of round-trip DMAs.
# Conv1 uses a host-built im2col (6-row stationary); conv9/16/17 use
# block-diagonal tap stationaries with parity-edge corrections.
# Orientation transposes are xbar DMAs split per batch so they pipeline
# with the producing/consuming compute.
#
# Layouts (i2 = chunk parity, ip = chunk pair, l = (2*ip+i2)*128 + c):
#   h-orient: [(i2,h)=128 part, (b=8, ip=8, c=128) free]   (convs, Wout, BN)
#   c-orient: [c=128 part, (b, ip, (i2,h)=128) free]       (per-h S4 matmuls)
#   state:    [(ri,n)=128 part, ...] re in partitions 0-63, im in 64-127

import numpy as np

import concourse.bass as bass
import concourse.bacc as bacc
import concourse.tile as tile
from concourse import mybir
from concourse.bass_utils import run_bass_kernel_spmd

F32 = mybir.dt.float32
F16 = mybir.dt.float16
AF = mybir.ActivationFunctionType
OP = mybir.AluOpType

NCORES = 8
B, H, N, L, NB = 64, 64, 64, 2048, 13
BL = B // NCORES          # 8 local batches
C = 128                   # chunk length
NCH = L // C              # 16 chunks
IP = NCH // 2             # 8 chunk pairs
KAP = 256.0               # state scaling to keep fp16 range
EPS = 1e-5
H2 = 2 * H                # 128 = (i2, h) partition extent
N2 = 2 * N                # 128 = (re/im, n) state extent
NSRC = 2 * IP - 1         # 15 qc source slots
HQ = 16                   # h per weight-stream quarter


# ---------------------------------------------------------------------------
# Host-side weight preprocessing (numpy) -> fp16 device matrices
# ---------------------------------------------------------------------------

def _host_prep(inputs):
    out = {}
    log_dt = np.asarray(inputs['s4_log_dt'], np.float64)
    logA_re = np.asarray(inputs['s4_logA_re'], np.float64)
    A_im = np.asarray(inputs['s4_A_im'], np.float64)
    C_re = np.asarray(inputs['s4_C_re'], np.float64)
    C_im = np.asarray(inputs['s4_C_im'], np.float64)
    D = np.asarray(inputs['s4_D'], np.float64)
    Wout = np.asarray(inputs['s4_Wout'], np.float64)
    bout = np.asarray(inputs['s4_bout'], np.float64)

    dt = np.exp(log_dt)[:, :, None]
    A = -np.exp(logA_re) + 1j * A_im
    dtA = dt * A
    w = np.exp(dtA)                                            # (NB,H,N)
    Ct = (C_re + 1j * C_im) * (np.exp(dtA) - 1.0) / A

    cc = np.arange(C)
    P = w[..., None] ** np.arange(2 * C + 1)                   # (NB,H,N,2C+1)
    K = 2.0 * np.real(np.einsum('jhn,jhne->jhe', Ct, P[..., :C]))
    K[:, :, 0] += D                                            # D*u folded

    # T lhsT [c', (h, c)] with T[c,c'] = K[c-c']
    dmat = cc[None, :] - cc[:, None]                           # (c',c)
    Tl = np.where((dmat >= 0)[None, None],
                  np.take_along_axis(np.broadcast_to(K[:, :, None, :],
                                                     (NB, H, C, C)),
                                     np.clip(dmat, 0, C - 1)[None, None],
                                     axis=3), 0.0)             # (NB,H,c',c)
    out['tmat'] = np.ascontiguousarray(
        Tl.transpose(2, 0, 1, 3).reshape(C, NB, H * C), np.float16)

    # V lhsT [c', (h, 2n)]: V[(ri,n),c'] = [Re;Im](Ct w^(C-1-c'))/KAP
    VC = Ct[..., None] * P[..., (C - 1) - cc]                  # (NB,H,N,c')
    Vl = np.concatenate([VC.real, VC.imag], axis=2) / KAP      # (NB,H,2N,c')
    out['vmat'] = np.ascontiguousarray(
        Vl.transpose(3, 0, 1, 2).reshape(C, NB, H * N2), np.float16)

    # ZV lhsT: same with an extra w^C factor (state pushed one more chunk)
    zC = w ** C
    VCz = zC[..., None] * VC
    ZVl = np.concatenate([VCz.real, VCz.imag], axis=2) / KAP
    out['zvmat'] = np.ascontiguousarray(
        ZVl.transpose(3, 0, 1, 2).reshape(C, NB, H * N2), np.float16)

    # M lhsT [(ri,n), (h, c)]: y_cross[c] = 2*KAP*[Re|-Im](w^{c+1}) . s
    MC = P[..., cc + 1]                                        # (NB,H,N,c)
    Ml = np.concatenate([2 * KAP * MC.real, -2 * KAP * MC.imag], axis=2)
    out['mmat'] = np.ascontiguousarray(
        Ml.transpose(0, 2, 1, 3).reshape(NB, N2, H * C).transpose(1, 0, 2),
        np.float16)

    # complex multipliers, BL-broadcast: tiles [(ri,n), NB, H, BL]
    def cmul_tiles(z):
        zr = np.broadcast_to(z.real.transpose(0, 2, 1)[:, :, :, None],
                             (NB, N, H, BL))
        zi = np.broadcast_to(z.imag.transpose(0, 2, 1)[:, :, :, None],
                             (NB, N, H, BL))
        r = np.concatenate([zr, zr], 1).transpose(1, 0, 2, 3)
        i = np.concatenate([zi, -zi], 1).transpose(1, 0, 2, 3)
        return (np.ascontiguousarray(r, np.float16),
                np.ascontiguousarray(i, np.float16))
    out['z2r'], out['z2i'] = cmul_tiles(w ** (2 * C))
    out['z1r'], out['z1i'] = cmul_tiles(zC)

    # Wout block-diag over i2: lhsT[(i2,h),(i2',o)] = Wout[o,h] d_{i2,i2'}
    wblk = np.zeros((NB, H2, H2))
    WT = Wout.transpose(0, 2, 1)
    wblk[:, :H, :H] = WT; wblk[:, H:, H:] = WT
    out['wblk'] = np.ascontiguousarray(wblk.transpose(1, 0, 2), np.float16)
    out['bout2'] = np.ascontiguousarray(
        np.concatenate([bout, bout], 1).T, np.float32)         # [128, NB]

    # conv stationaries (c9/c16/c17): per tap k block-diag over i2, plus
    # i2-crossing edge stationaries.
    def conv_stat(Wc):                                         # (O, Hin, 3)
        O = Wc.shape[0]; Hin = Wc.shape[1]
        res = {}
        for k in range(3):
            Wk = np.zeros((2 * Hin, 2 * O))
            Wk[:Hin, :O] = Wc[:, :, k].T; Wk[Hin:, O:] = Wc[:, :, k].T
            res[f'k{k}'] = Wk
        E0a = np.zeros((2 * Hin, 2 * O)); E0a[Hin:, :O] = Wc[:, :, 0].T
        E0b = np.zeros((2 * Hin, 2 * O)); E0b[:Hin, O:] = Wc[:, :, 0].T
        E2a = np.zeros((2 * Hin, 2 * O)); E2a[Hin:, :O] = Wc[:, :, 2].T
        E2b = np.zeros((2 * Hin, 2 * O)); E2b[:Hin, O:] = Wc[:, :, 2].T
        res.update(e0a=E0a, e0b=E0b, e2a=E2a, e2b=E2b)
        return {k_: v.astype(np.float16) for k_, v in res.items()}
    convall = np.zeros((128, 21, 128), np.float16)
    for ni, key in enumerate(('conv9_w', 'conv16_w', 'conv17_w')):
        cs = conv_stat(np.asarray(inputs[key], np.float64))
        for si, sfx in enumerate(('k0', 'k1', 'k2', 'e0a', 'e0b', 'e2a', 'e2b')):
            arr = cs[sfx]
            convall[:arr.shape[0], ni * 7 + si, :arr.shape[1]] = arr
    out['convall'] = np.ascontiguousarray(convall.reshape(128, 21 * 128))

    # conv1 im2col stationary [ (i2r, tap)=6, (i2, o)=128 ]
    W1 = np.asarray(inputs['conv1_w'], np.float64)             # (H, 1, 3)
    c1s = np.zeros((6, H2))
    for i2r in range(2):
        for tap in range(3):
            c1s[i2r * 3 + tap, i2r * H:(i2r + 1) * H] = W1[:, 0, tap]
    out['c1s'] = np.ascontiguousarray(c1s, np.float16)

    bn = np.zeros((H, 6), np.float32)
    for k, nm in enumerate(('bn1', 'bn9', 'bn16')):
        bn[:, 2 * k] = np.asarray(inputs[nm + '_g'], np.float32)
        bn[:, 2 * k + 1] = np.asarray(inputs[nm + '_b'], np.float32)
    out['bnp'] = bn

    # BN stat fold [128 -> 64] (mean of 16 core*half chunks) and channel
    # broadcast [64 -> 128] fp32 matrices
    ff = np.zeros((128, H), np.float32)
    ff[np.arange(128), np.arange(128) % H] = 1.0 / (2 * NCORES)
    out['ffold'] = ff
    f2 = np.zeros((H, 128), np.float32)
    f2[np.arange(128) % H, np.arange(128)] = 1.0
    out['f2'] = f2
    return out


def _host_prep_x(x_shard):
    # im2col for conv1: xim[(i2r,tap), b, ip, c] = x[b, (2ip+i2r)*128+c+tap-1]
    xs = np.asarray(x_shard, np.float32).reshape(BL, L)
    xp = np.zeros((BL, L + 2), np.float32)
    xp[:, 1:L + 1] = xs
    xim = np.zeros((6, BL, IP, C), np.float32)
    for i2r in range(2):
        for tap in range(3):
            sh = xp[:, tap:tap + L].reshape(BL, IP, 2, C)
            xim[i2r * 3 + tap] = sh[:, :, i2r, :]
    return np.ascontiguousarray(xim, np.float16)


# ---------------------------------------------------------------------------
# Device kernel
# ---------------------------------------------------------------------------

_CACHE = {}

CONV_SFX = ('k0', 'k1', 'k2', 'e0a', 'e0b', 'e2a', 'e2b')


def _bcast(zslice, n, at=1):
    """Insert a stride-0 axis of extent n into an AP after the partition."""
    ap = list(zslice.ap)
    ap2 = ap[:at] + [[0, n]] + ap[at:]
    return bass.AP(tensor=zslice.tensor, offset=zslice.offset, ap=ap2)


def _build():
    if 'nc' in _CACHE:
        return _CACHE['nc']
    import contextlib
    nc = bacc.Bacc("TRN2", target_bir_lowering=False, debug=False,
                   num_devices=NCORES)

    dram = {}
    def din(name, shape, dtype=F16):
        dram[name] = nc.dram_tensor(name, shape, dtype,
                                    kind="ExternalInput").ap()

    din('xim', [6, BL, IP, C])
    din('tmat', [C, NB, H * C]); din('vmat', [C, NB, H * N2])
    din('zvmat', [C, NB, H * N2]); din('mmat', [N2, NB, H * C])
    din('z2r', [N2, NB, H, BL]); din('z2i', [N2, NB, H, BL])
    din('z1r', [N2, NB, H, BL]); din('z1i', [N2, NB, H, BL])
    din('wblk', [H2, NB, H2]); din('bout2', [H2, NB], F32)
    din('convall', [128, 21 * 128])
    din('c1s', [6, H2])
    din('bnp', [H, 6], F32)
    din('ffold', [128, H], F32); din('f2', [H, 128], F32)
    out_d = nc.dram_tensor('out', [2, BL, IP, C], F32,
                           kind="ExternalOutput").ap()

    with tile.TileContext(nc) as tc:
        ctx = contextlib.ExitStack()
        P_stat = ctx.enter_context(tc.tile_pool(name="stat", bufs=1))
        P_act = ctx.enter_context(tc.tile_pool(name="act", bufs=1))
        P_wa = ctx.enter_context(tc.tile_pool(name="wa", bufs=3))
        P_wb = ctx.enter_context(tc.tile_pool(name="wb", bufs=3))
        P_z = ctx.enter_context(tc.tile_pool(name="wz", bufs=2))
        P_tmp = ctx.enter_context(tc.tile_pool(name="tmp", bufs=2))
        P_ps_a = ctx.enter_context(tc.tile_pool(name="psa", bufs=2,
                                                space="PSUM"))
        P_ps_b = ctx.enter_context(tc.tile_pool(name="psb", bufs=3,
                                                space="PSUM"))
        P_ps_w = ctx.enter_context(tc.tile_pool(name="psw", bufs=2,
                                                space="PSUM"))
        P_dram = ctx.enter_context(tc.tile_pool(name="cdram", bufs=1,
                                                space="DRAM"))

        # ---- warmup collective (pays one-time CC setup off-path) ----
        wu_in = P_dram.tile([128, 2], F32, tag="wuin")
        wu_out = P_dram.tile([128, 2], F32, tag="wuout", addr_space="Shared")
        wu_s = P_stat.tile([128, 2], F32, tag="wus")
        nc.vector.memset(wu_s, 0.0)
        nc.sync.dma_start(out=wu_in, in_=wu_s)
        nc.gpsimd.collective_compute("AllReduce", OP.add, ins=[wu_in[:]],
                                     outs=[wu_out[:]],
                                     replica_groups=[list(range(NCORES))])

        # ---- static tiles ----
        wblk_s = P_stat.tile([H2, NB, H2], F16)
        nc.sync.dma_start(out=wblk_s, in_=dram['wblk'])
        bout_s = P_stat.tile([H2, NB], F32)
        nc.sync.dma_start(out=bout_s, in_=dram['bout2'])
        bnp_s = P_stat.tile([H, 6], F32)
        nc.sync.dma_start(out=bnp_s, in_=dram['bnp'])
        xim_s = P_stat.tile([6, BL, IP, C], F16)
        nc.sync.dma_start(out=xim_s, in_=dram['xim'])
        c1s_s = P_stat.tile([6, H2], F16)
        nc.sync.dma_start(out=c1s_s, in_=dram['c1s'])
        ffold_s = P_stat.tile([128, H], F32)
        nc.sync.dma_start(out=ffold_s, in_=dram['ffold'])
        f2_s = P_stat.tile([H, 128], F32)
        nc.sync.dma_start(out=f2_s, in_=dram['f2'])
        convall_s = P_stat.tile([128, 21, 128], F16)
        nc.sync.dma_start(out=convall_s, in_=dram['convall'].rearrange(
            "p (k c) -> p k c", k=21))
        convw = {}
        for ni, nm in enumerate(('c9', 'c16', 'c17')):
            ncol = 2 if nm == 'c17' else H2
            convw[nm] = {sfx: convall_s[0:H2, ni * 7 + si, 0:ncol]
                         for si, sfx in enumerate(CONV_SFX)}

        # ---- activations ----
        u16c = P_act.tile([C, BL, IP, H2], F16, tag="u16c")
        g16c = P_act.tile([C, BL, IP, H2], F16, tag="g16c")
        gT = P_act.tile([H2, BL, IP, C], F16, tag="gT")
        uT = P_act.tile([H2, BL, IP, C], F16, tag="uT")
        uTn = P_act.tile([H2, BL, IP, C], F16, tag="uTn")
        sle_t = P_act.tile([N2, IP, H, BL], F16, tag="sle")
        pi_t = P_act.tile([N2, IP, H, BL], F16, tag="pi")
        qc_t = P_act.tile([N2, NSRC, H, BL], F16, tag="qc")
        gb128 = P_stat.tile([H2, 2], F32, tag="gb128")
        stats = P_stat.tile([H2, 16, 6], F32, tag="stats")
        bnscr = P_stat.tile([H2, 24], F32, tag="bnscr")

        u5 = u16c.rearrange("c b i (x h) -> c b i x h", x=2)
        qcp = qc_t[:, 1:NSRC].rearrange("n (m t) h b -> n m t h b", t=2)

        # ------------------------------------------------------------------
        def s4_block(j, u_h, u_next, last):
            z2r_s = P_z.tile([N2, H, BL], F16, tag="z2r")
            nc.sync.dma_start(out=z2r_s, in_=dram['z2r'][:, j])
            z2i_s = P_z.tile([N2, H, BL], F16, tag="z2i")
            nc.sync.dma_start(out=z2i_s, in_=dram['z2i'][:, j])
            z1r_s = P_z.tile([N2, H, BL], F16, tag="z1r")
            nc.sync.dma_start(out=z1r_s, in_=dram['z1r'][:, j])
            z1i_s = P_z.tile([N2, H, BL], F16, tag="z1i")
            nc.sync.dma_start(out=z1i_s, in_=dram['z1i'][:, j])

            # ---- A phase: sle + pi (V, ZV), 4h psum groups ----
            def a_phase(half):
                for q in (2 * half, 2 * half + 1):
                    hq = slice(HQ * q, HQ * (q + 1))
                    vm = P_wa.tile([C, HQ, N2], F16, tag="vm")
                    nc.sync.dma_start(
                        out=vm, in_=dram['vmat'][:, j].rearrange(
                            "p (h n) -> p h n", h=H)[:, hq, :])
                    zvm = P_wa.tile([C, HQ, N2], F16, tag="zvm")
                    nc.sync.dma_start(
                        out=zvm, in_=dram['zvmat'][:, j].rearrange(
                            "p (h n) -> p h n", h=H)[:, hq, :])
                    for g4 in range(HQ // 4):
                        acc = P_ps_a.tile([N2, 2, IP, 4, BL], F32,
                                          tag="apsum")
                        for hh in range(4):
                            hl = 4 * g4 + hh
                            h = HQ * q + hl
                            mv = u5[:, :, :, :, h].rearrange(
                                "c b i x -> c x i b")
                            nc.tensor.matmul(acc[:, :, :, hh, :],
                                             vm[:, hl, :], mv,
                                             start=True, stop=False)
                            mve = u5[:, :, :, 0, h].rearrange(
                                "c b i -> c i b")
                            nc.tensor.matmul(acc[:, 1, :, hh, :],
                                             zvm[:, hl, :], mve,
                                             start=False, stop=True)
                        hs4 = slice(HQ * q + 4 * g4, HQ * q + 4 * g4 + 4)
                        nc.scalar.activation(out=sle_t[:, :, hs4, :],
                                             in_=acc[:, 0], func=AF.Copy)
                        nc.scalar.activation(out=pi_t[:, :, hs4, :],
                                             in_=acc[:, 1], func=AF.Copy)

            # ---- DVE: carry scan + q construction per h-half ----
            def scan_phase(half):
                hs = slice(32 * half, 32 * (half + 1))
                nc.vector.tensor_copy(out=qc_t[:, 1, hs, :],
                                      in_=pi_t[:, 0, hs, :])
                nc.vector.tensor_copy(out=qc_t[:, 0, hs, :],
                                      in_=sle_t[:, 0, hs, :])
                z2r_h = z2r_s[:, hs, :]; z2i_h = z2i_s[:, hs, :]
                for m in range(1, IP - 1):
                    src = qc_t[:, 2 * m - 1, hs, :]
                    tsw = P_tmp.tile([N2, 32, BL], F16, tag="tsw")
                    nc.vector.tensor_tensor(out=tsw[0:N], in0=src[N:],
                                            in1=z2i_h[N:], op=OP.mult)
                    nc.vector.tensor_tensor(out=tsw[N:], in0=src[0:N],
                                            in1=z2i_h[0:N], op=OP.mult)
                    tzr = P_tmp.tile([N2, 32, BL], F16, tag="tzr")
                    nc.vector.tensor_tensor(out=tzr, in0=src, in1=z2r_h,
                                            op=OP.mult)
                    nc.vector.tensor_tensor(out=tzr, in0=tzr, in1=tsw,
                                            op=OP.add)
                    nc.vector.tensor_tensor(out=qc_t[:, 2 * m + 1, hs, :],
                                            in0=tzr, in1=pi_t[:, m, hs, :],
                                            op=OP.add)
                # q[m] = sle[m] + z1 (.) c[m-1], m = 1..7 in one sweep
                csrc = qcp[:, :, 0, hs, :]                     # c[0..6]
                qdst = qcp[:, :, 1, hs, :]                     # q[1..7]
                z1r_b = _bcast(z1r_s[:, hs, :], IP - 1)
                z1i_lo = _bcast(z1i_s[0:N, hs, :], IP - 1)
                z1i_hi = _bcast(z1i_s[N:, hs, :], IP - 1)
                tswq = P_tmp.tile([N2, IP - 1, 32, BL], F16, tag="tswq",
                                  bufs=1)
                nc.vector.tensor_tensor(out=tswq[0:N], in0=csrc[N:],
                                        in1=z1i_hi, op=OP.mult)
                nc.vector.tensor_tensor(out=tswq[N:], in0=csrc[0:N],
                                        in1=z1i_lo, op=OP.mult)
                tzrq = P_tmp.tile([N2, IP - 1, 32, BL], F16, tag="tzrq",
                                  bufs=1)
                nc.vector.tensor_tensor(out=tzrq, in0=csrc, in1=z1r_b,
                                        op=OP.mult)
                nc.vector.tensor_tensor(out=tzrq, in0=tzrq, in1=tswq,
                                        op=OP.add)
                nc.vector.tensor_tensor(out=qdst, in0=tzrq,
                                        in1=sle_t[:, 1:IP, hs, :], op=OP.add)

            # ---- B phase: y = T@u + M@qc, gelu evict ----
            def b_phase(half):
                for q in (2 * half, 2 * half + 1):
                    hq = slice(HQ * q, HQ * (q + 1))
                    tm = P_wb.tile([C, HQ, C], F16, tag="tm")
                    nc.sync.dma_start(
                        out=tm, in_=dram['tmat'][:, j].rearrange(
                            "p (h c) -> p h c", h=H)[:, hq, :])
                    mm = P_wb.tile([N2, HQ, C], F16, tag="mm")
                    nc.sync.dma_start(
                        out=mm, in_=dram['mmat'][:, j].rearrange(
                            "p (h c) -> p h c", h=H)[:, hq, :])
                    for g4 in range(HQ // 4):
                        acc = P_ps_b.tile([C, BL, IP, 2, 4], F32,
                                          tag="ypsum")
                        accf = acc.rearrange("c b i x h -> c b (i x) h")
                        for hh in range(4):
                            hl = 4 * g4 + hh
                            h = HQ * q + hl
                            nc.tensor.matmul(acc[:, :, :, :, hh],
                                             tm[:, hl, :],
                                             u5[:, :, :, :, h],
                                             start=True, stop=False)
                            qcm = qc_t[:, :, h, :].rearrange("n s b -> n b s")
                            nc.tensor.matmul(accf[:, :, 1:NCH, hh],
                                             mm[:, hl, :], qcm,
                                             start=False, stop=True)
                        sl = slice(HQ * q + 4 * g4, HQ * q + 4 * g4 + 4)
                        dst = g16c.rearrange(
                            "c b i (x h) -> c (b i) x h", x=2)[:, :, :, sl]
                        nc.scalar.activation(
                            out=dst,
                            in_=acc.rearrange("c b i x h -> c (b i) x h"),
                            func=AF.Gelu)

            a_phase(0)
            scan_phase(0)
            a_phase(1)
            scan_phase(1)
            b_phase(0)
            b_phase(1)

            # ---- per-batch: gT transpose, Wout + bout + residual, u16c ----
            gT_f = gT.rearrange("p b i c -> p (b i c)")
            uh_f = u_h.rearrange("p b i c -> p (b i c)")
            un_f = u_next.rearrange("p b i c -> p (b i c)")
            for b in range(BL):
                nc.sync.dma_start_transpose(
                    gT[:, b], g16c[:, b].rearrange("c i p -> c (i p)"))
                for t in range(2):
                    sl = slice(1024 * b + 512 * t, 1024 * b + 512 * (t + 1))
                    acc = P_ps_w.tile([H2, 512], F32, tag="wpsum")
                    nc.tensor.matmul(acc, wblk_s[:, j, :], gT_f[:, sl],
                                     start=True, stop=True)
                    nc.vector.scalar_tensor_tensor(
                        out=un_f[:, sl], in0=acc, scalar=bout_s[:, j:j + 1],
                        in1=uh_f[:, sl], op0=OP.add, op1=OP.add)
                if not last:
                    nc.sync.dma_start_transpose(
                        u16c[:, b], u_next[:, b].rearrange("p i c -> p (i c)"))

        # ------------------------------------------------------------------
        def conv_layer(src, w, big_out):
            """3-tap conv from h-orient src into gT raw + stats (big_out)
            or the final output DMA (conv17)."""
            for b in range(BL):
                for iph in range(2):
                    ip0 = 4 * iph
                    ips = slice(ip0, ip0 + 4)
                    acc = P_ps_b.tile([H2, 4, C], F32, tag="ypsum")
                    a = acc if big_out else acc[0:2]
                    nc.tensor.matmul(a, w['k1'], src[:, b, ips, :],
                                     start=True, stop=False)
                    nc.tensor.matmul(a[:, :, 1:C], w['k0'],
                                     src[:, b, ips, 0:C - 1],
                                     start=False, stop=False)
                    nc.tensor.matmul(a[:, :, 0:C - 1], w['k2'],
                                     src[:, b, ips, 1:C],
                                     start=False, stop=False)
                    if ip0 == 0:
                        nc.tensor.matmul(a[:, 1:4, 0:1], w['e0a'],
                                         src[:, b, 0:3, C - 1:C],
                                         start=False, stop=False)
                    else:
                        nc.tensor.matmul(a[:, 0:4, 0:1], w['e0a'],
                                         src[:, b, ip0 - 1:ip0 + 3, C - 1:C],
                                         start=False, stop=False)
                    nc.tensor.matmul(a[:, 0:4, 0:1], w['e0b'],
                                     src[:, b, ips, C - 1:C],
                                     start=False, stop=False)
                    nc.tensor.matmul(a[:, 0:4, C - 1:C], w['e2a'],
                                     src[:, b, ips, 0:1],
                                     start=False, stop=False)
                    if ip0 == 0:
                        nc.tensor.matmul(a[:, 0:4, C - 1:C], w['e2b'],
                                         src[:, b, 1:5, 0:1],
                                         start=False, stop=True)
                    else:
                        nc.tensor.matmul(a[:, 0:3, C - 1:C], w['e2b'],
                                         src[:, b, ip0 + 1:ip0 + 4, 0:1],
                                         start=False, stop=True)
                    if big_out:
                        nc.vector.bn_stats(out=stats[:, 2 * b + iph, :],
                                           in_=acc.rearrange(
                                               "p a c -> p (a c)"))
                        nc.scalar.activation(out=gT[:, b, ips, :], in_=acc,
                                             func=AF.Copy)
                    else:
                        ev = P_tmp.tile([2, 4, C], F32, tag="finev")
                        nc.scalar.activation(out=ev, in_=acc[0:2],
                                             func=AF.Copy)
                        nc.sync.dma_start(out=out_d[:, b, ips, :], in_=ev)

        def conv1_layer():
            for b in range(BL):
                for iph in range(2):
                    ips = slice(4 * iph, 4 * iph + 4)
                    acc = P_ps_b.tile([H2, 4, C], F32, tag="ypsum")
                    nc.tensor.matmul(acc, c1s_s, xim_s[:, b, ips, :],
                                     start=True, stop=True)
                    nc.vector.bn_stats(out=stats[:, 2 * b + iph, :],
                                       in_=acc.rearrange("p a c -> p (a c)"))
                    nc.scalar.activation(out=gT[:, b, ips, :], in_=acc,
                                         func=AF.Copy)

        def bn_finalize(k):
            """stats --bn_aggr/AllReduce/fold--> gb128 [128,2]=(g', b')."""
            mv = bnscr[:, 0:2]; pay = bnscr[:, 2:4]; red = bnscr[:, 4:6]
            nc.vector.bn_aggr(out=mv, in_=stats)
            nc.vector.tensor_tensor(out=pay[:, 1:2], in0=mv[:, 0:1],
                                    in1=mv[:, 0:1], op=OP.mult)
            nc.vector.tensor_tensor(out=pay[:, 1:2], in0=pay[:, 1:2],
                                    in1=mv[:, 1:2], op=OP.add)
            nc.vector.tensor_copy(out=pay[:, 0:1], in_=mv[:, 0:1])
            cin = P_dram.tile([H2, 2], F32, tag=f"bnin{k}")
            cout = P_dram.tile([H2, 2], F32, tag=f"bnout{k}",
                               addr_space="Shared")
            nc.sync.dma_start(out=cin, in_=pay)
            nc.gpsimd.collective_compute("AllReduce", OP.add, ins=[cin[:]],
                                         outs=[cout[:]],
                                         replica_groups=[list(range(NCORES))])
            nc.sync.dma_start(out=red, in_=cout)
            # fold i2-halves: gm [64, 2] = ffold.T @ red  (fp32 matmul)
            gacc = P_ps_w.tile([H, 2], F32, tag="wpsum")
            nc.tensor.matmul(gacc, ffold_s, red, start=True, stop=True)
            gm = bnscr[0:H, 6:8]
            nc.vector.tensor_copy(out=gm, in_=gacc)
            var = bnscr[0:H, 8:9]; eps_t = bnscr[0:H, 9:10]
            std = bnscr[0:H, 10:11]; rs = bnscr[0:H, 11:12]
            gp = bnscr[0:H, 12:14]
            nc.vector.tensor_tensor(out=var, in0=gm[:, 0:1], in1=gm[:, 0:1],
                                    op=OP.mult)
            nc.vector.tensor_tensor(out=var, in0=gm[:, 1:2], in1=var,
                                    op=OP.subtract)
            nc.vector.memset(eps_t, EPS)
            nc.scalar.activation(out=std, in_=var, func=AF.Sqrt, bias=eps_t)
            nc.vector.reciprocal(out=rs, in_=std)
            nc.vector.tensor_tensor(out=gp[:, 0:1],
                                    in0=bnp_s[:, 2 * k:2 * k + 1],
                                    in1=rs, op=OP.mult)
            nc.vector.tensor_tensor(out=gp[:, 1:2], in0=gp[:, 0:1],
                                    in1=gm[:, 0:1], op=OP.mult)
            nc.vector.tensor_tensor(out=gp[:, 1:2],
                                    in0=bnp_s[:, 2 * k + 1:2 * k + 2],
                                    in1=gp[:, 1:2], op=OP.subtract)
            # broadcast 64 -> 128 partitions: gb128 = f2.T @ gp
            bacc2 = P_ps_w.tile([H2, 2], F32, tag="wpsum")
            nc.tensor.matmul(bacc2, f2_s, gp, start=True, stop=True)
            nc.scalar.activation(out=gb128, in_=bacc2, func=AF.Copy)

        def bn_apply(dst):
            """dst = relu(g'*gT + b'), split scalar/vector, then u16c."""
            dst_f = dst.rearrange("p b i c -> p (b i c)")
            gT_ff = gT.rearrange("p b i c -> p (b i c)")
            for quad in range(4):
                sl = slice(2048 * quad, 2048 * (quad + 1))
                if quad % 2 == 0:
                    nc.scalar.activation(out=dst_f[:, sl], in_=gT_ff[:, sl],
                                         func=AF.Relu, bias=gb128[:, 1:2],
                                         scale=gb128[:, 0:1])
                else:
                    nc.vector.tensor_scalar(dst_f[:, sl], gT_ff[:, sl],
                                            gb128[:, 0:1], gb128[:, 1:2],
                                            OP.mult, OP.add)
                    nc.vector.tensor_scalar_max(dst_f[:, sl], dst_f[:, sl],
                                                0.0)
                for b in (2 * quad, 2 * quad + 1):
                    nc.sync.dma_start_transpose(
                        u16c[:, b], dst[:, b].rearrange("p i c -> p (i c)"))

        # ---------------- network ----------------
        conv1_layer()
        bn_finalize(0)
        bn_apply(uT)
        cur, nxt = uT, uTn
        for j in range(7):
            s4_block(j, cur, nxt, j == 6)
            cur, nxt = nxt, cur
        conv_layer(cur, convw['c9'], True)
        bn_finalize(1)
        bn_apply(cur)
        for j in range(7, 13):
            s4_block(j, cur, nxt, j == 12)
            cur, nxt = nxt, cur
        conv_layer(cur, convw['c16'], True)
        bn_finalize(2)
        bn_apply(cur)
        conv_layer(cur, convw['c17'], False)
        ctx.close()

    nc.compile()
    _CACHE['nc'] = nc
    return nc


# ---------------------------------------------------------------------------
# Entry point
# ---------------------------------------------------------------------------

def kernel(**inputs):
    nc = _build()
    prep = _host_prep(inputs)
    x = np.asarray(inputs['x'], np.float32)
    in_maps = []
    for c in range(NCORES):
        m = dict(prep)
        m['xim'] = _host_prep_x(x[c * BL:(c + 1) * BL])
        in_maps.append(m)
    res = run_bass_kernel_spmd(nc, in_maps, core_ids=list(range(NCORES)))
    outs = []
    for c in range(NCORES):
        o = res.results[c]['out']              # [2, BL, IP, C]
        outs.append(o.transpose(1, 2, 0, 3).reshape(BL, 1, L))
    return np.ascontiguousarray(np.concatenate(outs, 0), np.float32)
